# revision 20
# baseline (speedup 1.0000x reference)
"""Trainium2 Bass kernel for nn_CrossModeAttention (B=4, N=1024, D1=D2=512,
C=512, H=8, DH=64, 3 cross-attention layers sharing one softmax matrix).

Sharding: cores i and i+4 PAIR UP on batch element i, splitting the QUERY
tokens in half (core i: global tokens 0-511, core i+4: 512-1023). Keys /
values / the attention contraction stay in GLOBAL token order on both
cores, so the program is fully SPMD-symmetric: the only per-core
difference is which x1/x2 column-halves the host passes as the "own
query" tensors x1q/x2q, and which output rows the host reads back.

Per core: Q projection, QK^T, exp, AV and LayerNorm cover only 512
queries (halving the dominant ACT-exp cost and the per-layer epilogue
chains); K/V projections are duplicated (cheap, hidden under exp). After
each non-final layer the 4 locally-normalized token blocks are exchanged
with the pair core via a 2-rank AllGather (fp8, 256KB->512KB) in global
block order; both cores then re-read all 8 blocks uniformly.

Key algebraic facts exploited:
  - Q/K fixed across layers -> softmax(QK^T*scale) computed once (fp8).
  - Logits bounded (~1) -> exp without max-subtraction; mask is a no-op.
  - softmax denominators are layer-invariant: the colsum comes from a
    ones-column in the layer-0 AV moving operand only; layers 1-2 use
    64-wide V blocks and the saved reciprocals.
  - LN rstd = exp(-0.5*ln(var+eps)) so all ACTIVATEs share one table set.
"""

import numpy as np

import concourse.bass as bass
from concourse import mybir
from concourse.bass_utils import run_bass_kernel_spmd
from concourse.tile import TileContext

B, N, D, C, H, DH = 4, 1024, 512, 512, 8, 64
MB = N // 128               # 8 key blocks (global, both cores)
NQ = N // 2                 # 512 own queries per core
NBQ = NQ // 128             # 4 own query blocks
CC = C // 128               # 4 c-chunks = head pairs
KD = D // 128               # 4 contraction tiles per 512
LAYERS = 3
SCALE = float(D) ** -0.5
LN_EPS = 1e-5
GROUPS = [[0, 4], [1, 5], [2, 6], [3, 7]]

F32 = mybir.dt.float32
BF16 = mybir.dt.bfloat16
FP8 = mybir.dt.float8e4
AF = mybir.ActivationFunctionType
ALU = mybir.AluOpType


def _split_sync_waits(nc: bass.Bass) -> None:
    """Walrus codegen for this target accepts at most ONE sync wait per
    instruction, but Tile's scheduler may attach several (one per producer
    engine/DMA-queue clock). Insert single-wait NOPs on the same engine
    immediately before any multi-wait instruction — per-engine program
    order makes this semantically identical."""
    k = 0
    for f in nc.m.functions:
        for bb in f.blocks:
            newl = []
            changed = False
            for inst in bb.instructions:
                si = inst.sync_info
                waits = list(si.on_wait) if si and si.on_wait else []
                if len(waits) > 1:
                    for w in waits[:-1]:
                        nop = mybir.InstNoOp(name=f"WSPLIT-{k}", ins=[], outs=[])
                        k += 1
                        nop.engine = inst.engine
                        nop.sync_info = mybir.SyncInfo(on_wait=[w], on_update=[])
                        newl.append(nop)
                    si.on_wait = waits[-1:]
                    changed = True
                newl.append(inst)
            if changed:
                bb.instructions = newl


def _bcast(ap: bass.AP, count: int) -> bass.AP:
    """Append a step-0 (broadcast) free dimension of `count` to an AP."""
    return bass.AP(tensor=ap.tensor, offset=ap.offset, ap=[*ap.ap, [0, count]])


def build_kernel(with_gamma_beta: bool) -> bass.Bass:
    nc = bass.Bass(num_devices=8)

    x1t = nc.dram_tensor("x1t", [D, N], BF16, kind="ExternalInput")
    x2t = nc.dram_tensor("x2t", [D, N], BF16, kind="ExternalInput")
    x1q = nc.dram_tensor("x1q", [D, NQ], BF16, kind="ExternalInput")
    x2q = nc.dram_tensor("x2q", [D, NQ], BF16, kind="ExternalInput")
    wqt = nc.dram_tensor("wqt", [D, C], BF16, kind="ExternalInput")
    wkt = nc.dram_tensor("wkt", [D, C], BF16, kind="ExternalInput")
    wvt = nc.dram_tensor("wvt", [2 * D, C], BF16, kind="ExternalInput")
    if with_gamma_beta:
        gamma = nc.dram_tensor("gamma", [C], F32, kind="ExternalInput")
        beta = nc.dram_tensor("beta", [C], F32, kind="ExternalInput")
    out = nc.dram_tensor("out", [NQ, C], F32, kind="ExternalOutput")
    # AllGather staging (fp8 LN outputs, global block order)
    agin = [nc.dram_tensor(f"agin{l}", [NBQ, 128, C], FP8, kind="Internal")
            for l in range(LAYERS - 1)]
    agout = [nc.dram_tensor(f"agout{l}", [MB, 128, C], FP8, kind="Internal")
             for l in range(LAYERS - 1)]
    wmin = nc.dram_tensor("wmin", [128, 16], FP8, kind="Internal")
    wmout = nc.dram_tensor("wmout", [2, 128, 16], FP8, kind="Internal")

    with TileContext(nc) as tc:
        with tc.tile_pool(name="persist", bufs=1) as persist, \
             tc.tile_pool(name="qk", bufs=1) as qkpool, \
             tc.tile_pool(name="pp", bufs=1) as ppool, \
             tc.tile_pool(name="lay", bufs=2) as lay, \
             tc.tile_pool(name="v8", bufs=1) as v8, \
             tc.tile_pool(name="vb8", bufs=2) as vb8, \
             tc.tile_pool(name="gbp", bufs=1) as gbp, \
             tc.tile_pool(name="stat", bufs=8) as stat:
            Vf = persist.tile([128, NBQ, C], F32)
            rs = persist.tile([128, NBQ, H], F32)
            eps_t = persist.tile([128, 1], F32)
            nc.vector.memset(eps_t, LN_EPS)
            # tiny warmup AllGather: absorbs the first-collective ncfw
            # entry latency long before the layer-0 exchange needs it
            wm = persist.tile([128, 16], FP8)
            nc.vector.memset(wm, 0.0)
            nc.sync.dma_start(out=wmin[:], in_=wm)
            nc.gpsimd.collective_compute(
                "AllGather", ALU.bypass, replica_groups=GROUPS,
                ins=[wmin[:]], outs=[wmout[:]],
            )
            QT = qkpool.tile([128, CC, NQ], BF16)
            KT = qkpool.tile([128, CC, N], BF16)
            P = ppool.tile([128, H, MB // 2, 1024], FP8)

            # layer-0 AV moving operands (V cols + ones col), global blocks
            vaug = [v8.tile([128, H, 65], FP8, tag=f"va{mb}", name=f"va{mb}")
                    for mb in range(MB)]
            for mb in range(MB):
                nc.vector.memset(vaug[mb][:, :, 64], 1.0)
            if with_gamma_beta:
                gb = gbp.tile([128, C], F32)
                bb = gbp.tile([128, C], F32)
                g_ap = gamma[:]
                b_ap = beta[:]
                nc.sync.dma_start(
                    out=gb,
                    in_=bass.AP(tensor=g_ap.tensor, offset=0, ap=[[0, 128], *g_ap.ap]),
                )
                nc.sync.dma_start(
                    out=bb,
                    in_=bass.AP(tensor=b_ap.tensor, offset=0, ap=[[0, 128], *b_ap.ap]),
                )

            # ------- phase 1+2 interleaved: projections + A^T blocks + exp ----
            with tc.tile_pool(name="xs", bufs=1) as xs, \
                 tc.tile_pool(name="ws", bufs=1) as ws, \
                 tc.tile_pool(name="psqk", bufs=2, space="PSUM") as psqk, \
                 tc.tile_pool(name="psv", bufs=2, space="PSUM") as psvp, \
                 tc.tile_pool(name="psa", bufs=2, space="PSUM") as psa:
                wq = ws.tile([128, KD, C], BF16)
                wk = ws.tile([128, KD, C], BF16)
                wv = ws.tile([128, 2 * KD, C], BF16)
                x1sb = [xs.tile([128, N], BF16, name=f"x1_{t}") for t in range(KD)]
                x2sb = [xs.tile([128, N], BF16, name=f"x2_{t}") for t in range(KD)]
                x1qs = [xs.tile([128, NQ], BF16, name=f"x1q_{t}") for t in range(KD)]
                x2qs = [xs.tile([128, NQ], BF16, name=f"x2q_{t}") for t in range(KD)]
                # Batched DMAs (one 3D-AP call per tensor where possible),
                # issued from the idle GpSimd queue so the Sync engine's
                # serial descriptor generation doesn't gate startup.
                def _wload(dst, src, nt):
                    nc.gpsimd.dma_start(
                        out=dst,
                        in_=bass.AP(tensor=src, offset=0,
                                    ap=[[C, 128], [128 * C, nt], [1, C]]),
                    )
                _wload(wq, wqt, KD)
                _wload(wk, wkt, KD)
                for t in range(KD):
                    nc.gpsimd.dma_start(out=x1qs[t], in_=x1q[t * 128:(t + 1) * 128, :])
                for t in range(KD):
                    nc.gpsimd.dma_start(out=x2sb[t], in_=x2t[t * 128:(t + 1) * 128, :])
                for t in range(KD):
                    nc.gpsimd.dma_start(out=x1sb[t], in_=x1t[t * 128:(t + 1) * 128, :])
                _wload(wv, wvt, 2 * KD)
                for t in range(KD):
                    nc.gpsimd.dma_start(out=x2qs[t], in_=x2q[t * 128:(t + 1) * 128, :])

                def qk_chunks(cc):
                    """Q(cc) over own queries + K(cc) over all keys."""
                    def q_half(cc=cc):
                        ps = psqk.tile([128, 512], F32, tag="psqk")
                        for t in range(KD):
                            nc.tensor.matmul(
                                ps,
                                lhsT=wq[:, t, cc * 128:(cc + 1) * 128],
                                rhs=x1qs[t],
                                start=(t == 0), stop=(t == KD - 1),
                            )
                        nc.vector.tensor_copy(QT[:, cc, :], ps)
                    yield q_half
                    for nh in range(2):
                        def k_half(nh=nh, cc=cc):
                            ps = psqk.tile([128, 512], F32, tag="psqk")
                            for t in range(KD):
                                nc.tensor.matmul(
                                    ps,
                                    lhsT=wk[:, t, cc * 128:(cc + 1) * 128],
                                    rhs=x2sb[t][:, nh * 512:(nh + 1) * 512],
                                    start=(t == 0), stop=(t == KD - 1),
                                )
                            nc.vector.tensor_copy(
                                KT[:, cc, nh * 512:(nh + 1) * 512], ps
                            )
                        yield k_half

                def v_chunks(mbs):
                    """Global V blocks -> layer-0 vaug (fp8)."""
                    for mb in mbs:
                        def v_block(mb=mb):
                            ps = psvp.tile([128, C], F32, tag="psv")
                            for t in range(2 * KD):
                                x_sb = x1sb[t] if t < KD else x2sb[t - KD]
                                nc.tensor.matmul(
                                    ps,
                                    lhsT=x_sb[:, mb * 128:(mb + 1) * 128],
                                    rhs=wv[:, t, :],
                                    start=(t == 0), stop=(t == 2 * KD - 1),
                                )
                            nc.vector.tensor_copy(
                                vaug[mb][:, :, 0:64],
                                ps.rearrange("p (h d) -> p h d", d=DH),
                            )
                        yield v_block

                def vf_chunks(nbs):
                    """Own-query V blocks -> Vf (f32 residual)."""
                    for nb in nbs:
                        def vf_block(nb=nb):
                            ps = psvp.tile([128, C], F32, tag="psv")
                            for t in range(2 * KD):
                                x_sb = x1qs[t] if t < KD else x2qs[t - KD]
                                nc.tensor.matmul(
                                    ps,
                                    lhsT=x_sb[:, nb * 128:(nb + 1) * 128],
                                    rhs=wv[:, t, :],
                                    start=(t == 0), stop=(t == 2 * KD - 1),
                                )
                            nc.vector.tensor_copy(Vf[:, nb, :], ps)
                        yield vf_block

                def a_groups(cc):
                    """One psa group per (head, key-block-pair): 2 matmuls
                    + one 1024-wide exp over 2 key blocks x 512 queries.
                    Heads alternate so consecutive K=64 matmuls land in
                    different PE row groups."""
                    for g in range(MB // 2):
                        for hh in range(2):
                            def group(g=g, hh=hh):
                                pt = psa.tile([128, 1024], F32, tag="psa")
                                for j in range(2):
                                    mb = 2 * g + j
                                    nc.tensor.matmul(
                                        pt[:, j * 512:(j + 1) * 512],
                                        lhsT=KT[hh * 64:(hh + 1) * 64, cc, mb * 128:(mb + 1) * 128],
                                        rhs=QT[hh * 64:(hh + 1) * 64, cc, :],
                                        start=True, stop=True,
                                    )
                                nc.scalar.activation(
                                    out=P[:, 2 * cc + hh, g, :],
                                    in_=pt, func=AF.Exp, scale=SCALE,
                                )
                            yield group

                # Fill schedules: cc=0 carries only the next QK chunk (its
                # V inputs may still be in flight and would head-of-line
                # block the A-group matmuls behind them); V/Vf work shifts
                # to later cc rounds once all inputs have surely landed.
                fills = {
                    0: list(qk_chunks(1)),
                    1: list(v_chunks((0, 1))) + list(vf_chunks((0,)))
                       + list(qk_chunks(2)),
                    2: list(v_chunks((2, 3, 4))) + list(vf_chunks((1,)))
                       + list(qk_chunks(3)),
                    3: list(v_chunks((5, 6, 7))) + list(vf_chunks((2, 3))),
                }
                for chunk in qk_chunks(0):
                    chunk()
                for cc in range(CC):
                    fill = fills[cc]
                    groups = list(a_groups(cc))
                    k = 0
                    for gi, g in enumerate(groups):
                        g()
                        want = (gi + 1) * len(fill) // len(groups)
                        while k < want:
                            fill[k]()
                            k += 1

            # ---------------- phase 3: three AV + LayerNorm layers --------
            with tc.tile_pool(name="psl0", bufs=2, space="PSUM") as psl0, \
                 tc.tile_pool(name="psl", bufs=4, space="PSUM") as psl, \
                 tc.tile_pool(name="s8p", bufs=2) as s8p:
                for layer in range(LAYERS):
                    last = layer == LAYERS - 1
                    wid = 65 if layer == 0 else 64
                    for nb in range(NBQ):
                        if layer == 0:
                            T = psl0.tile([128, H, 128], F32, tag="T65")
                        else:
                            T = psl.tile([128, H, 64], F32, tag="T")
                        for h in range(H):
                            for mt in range(MB):
                                nc.tensor.matmul(
                                    T[:, h, 0:wid],
                                    lhsT=P[:, h, mt // 2,
                                           (mt % 2) * 512 + nb * 128:
                                           (mt % 2) * 512 + (nb + 1) * 128],
                                    rhs=(vaug[mt][:, h, 0:wid]
                                         if isinstance(vaug, list)
                                         else vaug[:, mt, h, :]),
                                    start=(mt == 0), stop=(mt == MB - 1),
                                )
                        if layer == 0:
                            nc.vector.reciprocal(rs[:, nb, :], T[:, :, 64])
                        y = lay.tile([128, C], F32, tag="y")
                        nc.vector.tensor_mul(
                            y.rearrange("p (h d) -> p h d", d=DH),
                            T[:, :, 0:64],
                            _bcast(rs[:, nb, :], DH),
                        )
                        nc.vector.tensor_add(y, y, Vf[:, nb, :])
                        st = stat.tile([128, 6], F32, tag="st")
                        mv = stat.tile([128, 2], F32, tag="mv")
                        nc.vector.bn_stats(st, y)
                        nc.vector.bn_aggr(mv, st)
                        # rstd = (var+eps)^-0.5 via ln/exp (same ACT table
                        # set as the attention exp -> no table reload)
                        lnv = stat.tile([128, 1], F32, tag="lnv")
                        rstd = stat.tile([128, 1], F32, tag="rstd")
                        nc.scalar.activation(
                            out=lnv, in_=mv[:, 1:2], func=AF.Ln,
                            bias=eps_t, scale=1.0,
                        )
                        nc.scalar.activation(
                            out=rstd, in_=lnv, func=AF.Exp, scale=-0.5,
                        )
                        if last:
                            dest = lay.tile([128, C], F32, tag="osb")
                        else:
                            dest = Vf[:, nb, :]
                        if with_gamma_beta:
                            tmp = lay.tile([128, C], F32, tag="tmp")
                            nc.vector.tensor_scalar(
                                tmp, y, scalar1=mv[:, 0:1], scalar2=rstd,
                                op0=ALU.subtract, op1=ALU.mult,
                            )
                            nc.vector.tensor_mul(tmp, tmp, gb)
                            nc.vector.tensor_add(dest, tmp, bb)
                        else:
                            nc.vector.tensor_scalar(
                                dest, y, scalar1=mv[:, 0:1], scalar2=rstd,
                                op0=ALU.subtract, op1=ALU.mult,
                            )
                        if last:
                            nc.sync.dma_start(
                                out=out[nb * 128:(nb + 1) * 128, :], in_=dest
                            )
                        else:
                            s8 = s8p.tile([128, C], FP8, tag="s8")
                            nc.scalar.copy(s8, dest)
                            nc.sync.dma_start(out=agin[layer][nb], in_=s8)
                    if not last:
                        nc.gpsimd.collective_compute(
                            "AllGather",
                            ALU.bypass,
                            replica_groups=GROUPS,
                            ins=[agin[layer][:]],
                            outs=[agout[layer][:]],
                        )
                        vn = vb8.tile([128, MB, H, 64], FP8, tag="vb",
                                      name=f"vn{layer}")
                        # agout is [mb, token, c]; SBUF wants token on the
                        # partition dim -> explicit AP with mb as a free dim
                        nc.sync.dma_start(
                            out=vn,
                            in_=bass.AP(tensor=agout[layer], offset=0,
                                        ap=[[C, 128], [128 * C, MB], [1, C]]),
                        )
                        vaug = vn

    _split_sync_waits(nc)
    return nc


_NPBF16 = mybir.dt.np(BF16)


def make_in_maps(x1, x2, Wq, Wk, Wv, g=None, bt=None):
    wqt = np.ascontiguousarray(np.asarray(Wq, np.float32).T).astype(_NPBF16)
    wkt = np.ascontiguousarray(np.asarray(Wk, np.float32).T).astype(_NPBF16)
    wvt = np.ascontiguousarray(np.asarray(Wv, np.float32).T).astype(_NPBF16)
    in_maps = []
    for i in range(8):
        b = i % B
        hi = i // B
        x1tb = np.ascontiguousarray(np.asarray(x1[b], np.float32).T).astype(_NPBF16)
        x2tb = np.ascontiguousarray(np.asarray(x2[b], np.float32).T).astype(_NPBF16)
        m = {
            "x1t": x1tb,
            "x2t": x2tb,
            "x1q": np.ascontiguousarray(x1tb[:, hi * NQ:(hi + 1) * NQ]),
            "x2q": np.ascontiguousarray(x2tb[:, hi * NQ:(hi + 1) * NQ]),
            "wqt": wqt,
            "wkt": wkt,
            "wvt": wvt,
        }
        if g is not None:
            m["gamma"] = g
            m["beta"] = bt
        in_maps.append(m)
    return in_maps


def kernel(x1, x2, Wq, Wk, Wv, ln_gamma, ln_beta):
    g = np.asarray(ln_gamma, np.float32)
    bt = np.asarray(ln_beta, np.float32)
    with_gb = not (np.all(g == 1.0) and np.all(bt == 0.0))

    in_maps = make_in_maps(x1, x2, Wq, Wk, Wv,
                           g if with_gb else None, bt if with_gb else None)
    nc = build_kernel(with_gb)
    res = run_bass_kernel_spmd(nc, in_maps, list(range(8)))
    return np.stack([
        np.concatenate([res.results[b]["out"], res.results[b + 4]["out"]], axis=0)
        for b in range(B)
    ]).astype(np.float32)
